# revision 15
# baseline (speedup 1.0000x reference)
"""BinaryConv2d (3x3, SAME, NHWC) Trainium2 Bass kernel — 1D Winograd F(2,3).

Strategy (v3):
  - Data-parallel over batch: 32 images -> 8 cores x 4 images. No collectives.
  - Host prep: x cast to bf16 (round-to-nearest); Wq = sign(W) combined with
    the width-direction Winograd filter transform G = [[1,0,0],[.5,.5,.5],
    [.5,-.5,.5],[0,0,1]] into wt[cin, dh, t, cout] bf16 (values in
    {0,+-0.5,+-1,+-1.5} — exact in bf16). Bias is added on the host (exact;
    b == 0 in the reference setup).
  - 1D Winograd F(2,3) along width; the 3 vertical taps stay direct and
    accumulate in PSUM: 12 matmul streams per 2 output pixels vs 18 direct.
  - Pipeline is a flat stream of 28 units (16 output rows each) and 196
    M-tiles (128 Winograd positions each, 7 per unit), prepped 2 units ahead:
      1. HWDGE transpose-DMA: x rows (r0-1 .. r0+16) bf16 [18*112, 128] ->
         SBUF xb [cin, 2016] channel-major; vertical pad rows memset 0.
      2. DVE width transform (3 shifted tensor ops + 2 strided edge fixups):
           e[r,j] = x[r,j-1] - x[r,j+1]   (horizontal SAME pads = 0)
           f[r,j] = x[r,j]   + x[r,j+1]
           g[r,j] = x[r,j+1] - x[r,j]
         V-phases: V0=e[2i], V1=f[2i], V2=g[2i], V3=e[2i+1].
      3. Per M-tile: positions p = 56*r + i flatten uniformly; lhsT(t,dh) =
         egf[s][2p + 112*dh (+1 for V3)] strided by 2 (M=128), rhs =
         wt[:, dh, t, :] (N=256), 4 t-groups x 3 dh accumulating matmuls
         into one 2-bank PSUM tile [128, 4, 256].
      4. One ACT (scalar engine) copy PSUM -> SBUF m bf16 per M-tile.
      5. DVE inverse transform, batched over M-tile pairs (FD=512, in-place
         second ops): y0 = (m0+m1)+m2, y1 = (m1-m2)-m3.
      6. SWDGE cast-DMA store per pair: y bf16 [128, 2, 2, 256] -> HBM f32
         NHWC; positions are row-major so each store is one linear 512KB
         range (pairs may span image boundaries; images are contiguous).
"""

import numpy as np

N_CORES = 8
H = 112
W_DIM = 112
CIN = 128
COUT = 256
BATCH = 32
IMG_PER_CORE = BATCH // N_CORES


def _build_program(n_img, h, w, cin, cout):
    import concourse.bacc as bacc
    import concourse.mybir as mybir
    import concourse.tile as tile

    f32 = mybir.dt.float32
    bf16 = mybir.dt.bfloat16

    nc = bacc.Bacc(
        "TRN2", target_bir_lowering=False, debug=False, num_devices=N_CORES
    )
    x_d = nc.dram_tensor("x", [n_img, h, w, cin], bf16, kind="ExternalInput").ap()
    wt_d = nc.dram_tensor("wt", [cin, 3, 4, cout], bf16, kind="ExternalInput").ap()
    out_d = nc.dram_tensor(
        "out", [n_img, h, w, cout], f32, kind="ExternalOutput"
    ).ap()

    RU = 16  # output rows per unit
    n_units_img = h // RU
    n_units = n_img * n_units_img
    XROWS = RU + 2  # input rows incl. vertical halo
    XL = XROWS * w  # 2016 flat elements per xb/egf buffer
    tiles_w = w // 2  # 56 F(2,3) tiles per output row
    MT_PER_UNIT = RU * tiles_w // 128  # 7 M-tiles of 128 positions
    n_mt = n_units * MT_PER_UNIT  # 196
    PREP_AHEAD = 2

    with tile.TileContext(nc) as tc:
        with (
            tc.tile_pool(name="consts", bufs=1) as cpool,
            tc.tile_pool(name="xb", bufs=3) as xbpool,
            tc.tile_pool(name="egf", bufs=4) as egfpool,
            tc.tile_pool(name="psum", bufs=4, space="PSUM") as pspool,
            tc.tile_pool(name="msb", bufs=8) as mpool,
            tc.tile_pool(name="yst", bufs=8) as ypool,
        ):
            wt_t = cpool.tile([cin, 3, 4, cout], bf16)
            # SWDGE, so HWDGE queue pacing can't delay unit 0's transpose
            nc.gpsimd.dma_start(out=wt_t[:], in_=wt_d[:])

            def prep_geom(gu, split):
                img, unit = divmod(gu, n_units_img)
                r_lo = unit * RU - 1
                r_hi = unit * RU + RU + 1
                lo = max(r_lo, 0)
                hi = min(r_hi, h)
                dst_off = (lo - r_lo) * w
                if split:
                    cuts = [lo, lo + 6, lo + 12, hi]
                    qs = [dst_off + (c - lo) * w for c in cuts]
                    bounds = [
                        (cuts[k], cuts[k + 1], (qs[k] - 1 if k else 0),
                         qs[k + 1] if k + 2 < len(cuts) else XL)
                        for k in range(len(cuts) - 1)
                    ]
                else:
                    bounds = [(lo, hi, 0, XL)]
                return img, r_lo, r_hi, lo, dst_off, bounds

            def prep_dma(gu, split=False, dq=None):
                img, r_lo, r_hi, lo, dst_off, bounds = prep_geom(gu, split)
                xb = xbpool.tile([cin, XL], bf16, tag="xb")
                if r_lo < 0:
                    nc.vector.memset(xb[:, 0:w], 0.0)
                if r_hi > h:
                    nc.vector.memset(xb[:, XL - w : XL], 0.0)
                for blo, bhi, _, _ in bounds:
                    doff = dst_off + (blo - lo) * w
                    (dq or nc.sync).dma_start(
                        out=xb[:, doff : doff + (bhi - blo) * w],
                        in_=x_d[img, blo:bhi].rearrange("r w c -> (r w) c"),
                        transpose=True,
                    )
                return xb

            def prep_mains(gu, xb, split=False):
                img, r_lo, r_hi, lo, dst_off, bounds = prep_geom(gu, split)
                egf = egfpool.tile([cin, 3, XL], bf16, tag="egf")
                ev = egf[:, 0, :].rearrange("p (r j) -> p r j", j=w)
                xv = xb.rearrange("p (r j) -> p r j", j=w)
                for bi, (blo, bhi, qa, qb) in enumerate(bounds):
                    doff = dst_off + (blo - lo) * w
                    # main shifted passes over this slab (contiguous, bf16 2x)
                    # (later slabs start e one element in: the boundary j=111
                    # element belongs to the previous slab's fixup)
                    ea = (qa + 1) if bi else max(qa, 1)
                    nc.vector.tensor_sub(
                        egf[:, 0, ea : qb - 1],
                        xb[:, ea - 1 : qb - 2],
                        xb[:, ea + 1 : qb],
                    )
                    nc.vector.tensor_add(
                        egf[:, 1, qa : qb - 1], xb[:, qa : qb - 1], xb[:, qa + 1 : qb]
                    )
                    nc.gpsimd.tensor_sub(
                        egf[:, 2, qa : qb - 1], xb[:, qa + 1 : qb], xb[:, qa : qb - 1]
                    )
                    # e edge fixups, slab rows only (so slab 1's M-tiles don't
                    # wait on slab 2): e[r,0] = -x[r,1]; e[r,111] = x[r,110]
                    r0 = doff // w if bi else 0
                    r1 = doff // w + (bhi - blo) if bi + 1 < len(bounds) else XROWS
                    nc.vector.tensor_scalar_mul(
                        ev[:, r0:r1, 0], xv[:, r0:r1, 1], -1.0
                    )
                    nc.vector.tensor_copy(
                        ev[:, r0:r1, w - 1], xv[:, r0:r1, w - 2]
                    )
                return egf

            # (slot in egf, parity) per Winograd t-phase
            TSEL = [(0, 0), (1, 0), (2, 0), (0, 1)]
            outv = out_d.rearrange("i h w c -> (i h w c)").rearrange(
                "(p x) -> p x", x=2 * cout
            )  # [25088, 512]: row P = output-pixel pair at position P

            egfs = {}
            xbs = {}
            for u in range(PREP_AHEAD + 1):
                xbs[u] = prep_dma(u, split=(u == 0))
                egfs[u] = prep_mains(u, xbs[u], split=(u == 0))

            pend = None  # (yst, m_pair) for an incomplete store pair
            for mt in range(n_mt):
                gu, mti = divmod(mt, MT_PER_UNIT)
                # staged lookahead: unit gu+P+1's transpose issues at mti==0,
                # its DVE transform at mti==5 — by then the transpose data has
                # landed, so the DVE FIFO head never blocks on a DMA while
                # ready inverse-transform work sits queued behind it
                nu = gu + PREP_AHEAD + 1
                if mti == 0 and nu < n_units:
                    xbs[nu] = prep_dma(nu)
                if mti == 5 and nu < n_units:
                    egfs[nu] = prep_mains(nu, xbs.pop(nu))
                    egfs.pop(gu - 1, None)
                egf = egfs[gu]
                p0 = mti * 128
                if pend is None:
                    yst = ypool.tile([128, 2, 2, cout], bf16, tag="y")
                    m = mpool.tile([128, 2, 4, cout], bf16, tag="m")
                    ab = 0
                else:
                    yst, m = pend
                    ab = 1
                ps = pspool.tile([128, 4, cout], f32, tag="ps")
                for t in range(4):
                    s, par = TSEL[t]
                    evw = egf[:, s, :].rearrange("p (x two) -> p x two", two=2)
                    for dh in range(3):
                        q0 = p0 + tiles_w * dh
                        nc.tensor.matmul(
                            ps[:, t, :],
                            evw[:, q0 : q0 + 128, par],
                            wt_t[:, dh, t, :],
                            start=(dh == 0),
                            stop=(dh == 2),
                        )
                if mt % 14 == 7:
                    nc.vector.tensor_copy(m[:, ab, :, :], ps[:])
                else:
                    nc.scalar.copy(m[:, ab, :, :], ps[:])
                if pend is None:
                    pend = (yst, m)
                else:
                    # inverse transform for both M-tiles of the pair (FD=512)
                    y0 = yst[:, :, 0, :]
                    y1 = yst[:, :, 1, :]
                    nc.vector.tensor_add(y0, m[:, :, 0, :], m[:, :, 1, :])
                    nc.vector.tensor_add(y0, y0, m[:, :, 2, :])
                    nc.vector.tensor_sub(y1, m[:, :, 1, :], m[:, :, 2, :])
                    nc.vector.tensor_sub(y1, y1, m[:, :, 3, :])
                    b0 = mt - 1  # pair covers global M-tiles mt-1, mt
                    dst = outv.rearrange("(b p) x -> b p x", p=128)[
                        b0 : b0 + 2
                    ].rearrange("b p x -> p b x")
                    nc.gpsimd.dma_start(
                        out=dst, in_=yst[:].rearrange("p b j c -> p b (j c)")
                    )
                    pend = None

    nc.compile()
    return nc


_cached_nc = None


def _get_program():
    global _cached_nc
    if _cached_nc is None:
        _cached_nc = _build_program(IMG_PER_CORE, H, W_DIM, CIN, COUT)
    return _cached_nc


def _prep_inputs(x, W):
    import ml_dtypes

    wq = np.sign(W.astype(np.float32))  # sign(0)=0 matches jnp.sign
    wt = np.empty((3, 4, CIN, COUT), np.float32)
    wt[:, 0] = wq[:, 0]
    wt[:, 1] = (wq[:, 0] + wq[:, 1] + wq[:, 2]) * 0.5
    wt[:, 2] = (wq[:, 0] - wq[:, 1] + wq[:, 2]) * 0.5
    wt[:, 3] = wq[:, 2]
    # [3,4,cin,cout] -> [cin, 3, 4, cout]; values exact in bf16
    wt = np.ascontiguousarray(wt.transpose(2, 0, 1, 3)).astype(
        ml_dtypes.bfloat16
    )
    xb = x.astype(ml_dtypes.bfloat16)  # round-to-nearest
    in_maps = []
    for c in range(N_CORES):
        xs = np.ascontiguousarray(xb[c * IMG_PER_CORE : (c + 1) * IMG_PER_CORE])
        in_maps.append({"x": xs, "wt": wt})
    return in_maps


def run(x, W, b, trace=False, tmpdir=None):
    from concourse import bass_utils

    if trace:
        # the agent image's antenv lacks axon_hooks; wire the NTFF profile
        # hook up manually so trace=True yields exec_time_ns + pftrace
        import sys, types

        if "antenv.axon_hooks" not in sys.modules:
            import antenv
            from trn_agent_boot.trn_boot import _ntff_profile_via_ctypes

            mod = types.ModuleType("antenv.axon_hooks")
            _hook = _ntff_profile_via_ctypes("/opt/axon/libaxon_pjrt.so")
            mod.get_axon_ntff_profile_hook = lambda: _hook
            sys.modules["antenv.axon_hooks"] = mod
            antenv.axon_hooks = mod

    nc = _get_program()
    in_maps = _prep_inputs(x, W)
    res = bass_utils.run_bass_kernel_spmd(
        nc, in_maps, list(range(N_CORES)), trace=trace, tmpdir=tmpdir
    )
    out = np.concatenate([res.results[i]["out"] for i in range(N_CORES)], axis=0)
    b = np.asarray(b, dtype=np.float32)
    if b.any():
        out = out + b  # exact; b == 0 in the reference setup
    return out, res


def kernel(x, W, b):
    out, _ = run(x, W, b, trace=False)
    return out


# revision 16
# speedup vs baseline: 1.2391x; 1.2391x over previous
"""BinaryConv2d (3x3, SAME, NHWC) Trainium2 Bass kernel — 1D Winograd F(2,3).

Strategy (v3):
  - Data-parallel over batch: 32 images -> 8 cores x 4 images. No collectives.
  - Host prep: x cast to bf16 (round-to-nearest); Wq = sign(W) combined with
    the width-direction Winograd filter transform G = [[1,0,0],[.5,.5,.5],
    [.5,-.5,.5],[0,0,1]] into wt[cin, dh, t, cout] bf16 (values in
    {0,+-0.5,+-1,+-1.5} — exact in bf16). Bias is added on the host (exact;
    b == 0 in the reference setup).
  - 1D Winograd F(2,3) along width; the 3 vertical taps stay direct and
    accumulate in PSUM: 12 matmul streams per 2 output pixels vs 18 direct.
  - Pipeline is a flat stream of 28 units (16 output rows each) and 196
    M-tiles (128 Winograd positions each, 7 per unit), prepped 2 units ahead:
      1. HWDGE transpose-DMA: x rows (r0-1 .. r0+16) bf16 [18*112, 128] ->
         SBUF xb [cin, 2016] channel-major; vertical pad rows memset 0.
      2. DVE width transform (3 shifted tensor ops + 2 strided edge fixups):
           e[r,j] = x[r,j-1] - x[r,j+1]   (horizontal SAME pads = 0)
           f[r,j] = x[r,j]   + x[r,j+1]
           g[r,j] = x[r,j+1] - x[r,j]
         V-phases: V0=e[2i], V1=f[2i], V2=g[2i], V3=e[2i+1].
      3. Per M-tile: positions p = 56*r + i flatten uniformly; lhsT(t,dh) =
         egf[s][2p + 112*dh (+1 for V3)] strided by 2 (M=128), rhs =
         wt[:, dh, t, :] (N=256), 4 t-groups x 3 dh accumulating matmuls
         into one 2-bank PSUM tile [128, 4, 256].
      4. One ACT (scalar engine) copy PSUM -> SBUF m bf16 per M-tile.
      5. DVE inverse transform, batched over M-tile pairs (FD=512, in-place
         second ops): y0 = (m0+m1)+m2, y1 = (m1-m2)-m3.
      6. SWDGE cast-DMA store per pair: y bf16 [128, 2, 2, 256] -> HBM f32
         NHWC; positions are row-major so each store is one linear 512KB
         range (pairs may span image boundaries; images are contiguous).
"""

import numpy as np

N_CORES = 8
H = 112
W_DIM = 112
CIN = 128
COUT = 256
BATCH = 32
IMG_PER_CORE = BATCH // N_CORES


def _build_program(n_img, h, w, cin, cout):
    import concourse.bacc as bacc
    import concourse.mybir as mybir
    import concourse.tile as tile

    f32 = mybir.dt.float32
    bf16 = mybir.dt.bfloat16

    nc = bacc.Bacc(
        "TRN2", target_bir_lowering=False, debug=False, num_devices=N_CORES
    )
    x_d = nc.dram_tensor("x", [n_img, h, w, cin], bf16, kind="ExternalInput").ap()
    wt_d = nc.dram_tensor("wt", [cin, 3, 4, cout], bf16, kind="ExternalInput").ap()
    out_d = nc.dram_tensor(
        "out", [n_img, h, w, cout], f32, kind="ExternalOutput"
    ).ap()

    RU = 16  # output rows per unit
    n_units_img = h // RU
    n_units = n_img * n_units_img
    XROWS = RU + 2  # input rows incl. vertical halo
    XL = XROWS * w  # 2016 flat elements per xb/egf buffer
    tiles_w = w // 2  # 56 F(2,3) tiles per output row
    MT_PER_UNIT = RU * tiles_w // 128  # 7 M-tiles of 128 positions
    n_mt = n_units * MT_PER_UNIT  # 196
    PREP_AHEAD = 2

    with tile.TileContext(nc) as tc:
        with (
            tc.tile_pool(name="consts", bufs=1) as cpool,
            tc.tile_pool(name="xb", bufs=3) as xbpool,
            tc.tile_pool(name="egf", bufs=4) as egfpool,
            tc.tile_pool(name="psum", bufs=4, space="PSUM") as pspool,
            tc.tile_pool(name="msb", bufs=8) as mpool,
            tc.tile_pool(name="yst", bufs=8) as ypool,
        ):
            wt_t = cpool.tile([cin, 3, 4, cout], bf16)
            # SWDGE, so HWDGE queue pacing can't delay unit 0's transpose
            nc.gpsimd.dma_start(out=wt_t[:], in_=wt_d[:])

            def prep_geom(gu, split):
                img, unit = divmod(gu, n_units_img)
                r_lo = unit * RU - 1
                r_hi = unit * RU + RU + 1
                lo = max(r_lo, 0)
                hi = min(r_hi, h)
                dst_off = (lo - r_lo) * w
                if split:
                    cuts = [lo, lo + 6, lo + 12, hi]
                    qs = [dst_off + (c - lo) * w for c in cuts]
                    bounds = [
                        (cuts[k], cuts[k + 1], (qs[k] - 1 if k else 0),
                         qs[k + 1] if k + 2 < len(cuts) else XL)
                        for k in range(len(cuts) - 1)
                    ]
                else:
                    bounds = [(lo, hi, 0, XL)]
                return img, r_lo, r_hi, lo, dst_off, bounds

            def prep_dma(gu, split=False, dq=None):
                img, r_lo, r_hi, lo, dst_off, bounds = prep_geom(gu, split)
                xb = xbpool.tile([cin, XL], bf16, tag="xb")
                if r_lo < 0:
                    nc.vector.memset(xb[:, 0:w], 0.0)
                if r_hi > h:
                    nc.vector.memset(xb[:, XL - w : XL], 0.0)
                for blo, bhi, _, _ in bounds:
                    doff = dst_off + (blo - lo) * w
                    (dq or nc.sync).dma_start(
                        out=xb[:, doff : doff + (bhi - blo) * w],
                        in_=x_d[img, blo:bhi].rearrange("r w c -> (r w) c"),
                        transpose=True,
                    )
                return xb

            def prep_mains(gu, xb, split=False):
                img, r_lo, r_hi, lo, dst_off, bounds = prep_geom(gu, split)
                egf = egfpool.tile([cin, 3, XL], bf16, tag="egf")
                ev = egf[:, 0, :].rearrange("p (r j) -> p r j", j=w)
                xv = xb.rearrange("p (r j) -> p r j", j=w)
                for bi, (blo, bhi, qa, qb) in enumerate(bounds):
                    doff = dst_off + (blo - lo) * w
                    # main shifted passes over this slab (contiguous, bf16 2x)
                    # (later slabs start e one element in: the boundary j=111
                    # element belongs to the previous slab's fixup)
                    ea = (qa + 1) if bi else max(qa, 1)
                    nc.vector.tensor_sub(
                        egf[:, 0, ea : qb - 1],
                        xb[:, ea - 1 : qb - 2],
                        xb[:, ea + 1 : qb],
                    )
                    nc.vector.tensor_add(
                        egf[:, 1, qa : qb - 1], xb[:, qa : qb - 1], xb[:, qa + 1 : qb]
                    )
                    nc.vector.tensor_sub(
                        egf[:, 2, qa : qb - 1], xb[:, qa + 1 : qb], xb[:, qa : qb - 1]
                    )
                    # e edge fixups, slab rows only (so slab 1's M-tiles don't
                    # wait on slab 2): e[r,0] = -x[r,1]; e[r,111] = x[r,110]
                    r0 = doff // w if bi else 0
                    r1 = doff // w + (bhi - blo) if bi + 1 < len(bounds) else XROWS
                    nc.vector.tensor_scalar_mul(
                        ev[:, r0:r1, 0], xv[:, r0:r1, 1], -1.0
                    )
                    nc.vector.tensor_copy(
                        ev[:, r0:r1, w - 1], xv[:, r0:r1, w - 2]
                    )
                return egf

            # (slot in egf, parity) per Winograd t-phase
            TSEL = [(0, 0), (1, 0), (2, 0), (0, 1)]
            outv = out_d.rearrange("i h w c -> (i h w c)").rearrange(
                "(p x) -> p x", x=2 * cout
            )  # [25088, 512]: row P = output-pixel pair at position P

            egfs = {}
            xbs = {}
            for u in range(PREP_AHEAD + 1):
                xbs[u] = prep_dma(u, split=(u == 0))
                egfs[u] = prep_mains(u, xbs[u], split=(u == 0))

            pend = None  # (yst, m_pair) for an incomplete store pair
            for mt in range(n_mt):
                gu, mti = divmod(mt, MT_PER_UNIT)
                # staged lookahead: unit gu+P+1's transpose issues at mti==0,
                # its DVE transform at mti==5 — by then the transpose data has
                # landed, so the DVE FIFO head never blocks on a DMA while
                # ready inverse-transform work sits queued behind it
                nu = gu + PREP_AHEAD + 1
                if mti == 0 and nu < n_units:
                    xbs[nu] = prep_dma(nu)
                if mti == 5 and nu < n_units:
                    egfs[nu] = prep_mains(nu, xbs.pop(nu))
                    egfs.pop(gu - 1, None)
                egf = egfs[gu]
                p0 = mti * 128
                if pend is None:
                    yst = ypool.tile([128, 2, 2, cout], bf16, tag="y")
                    m = mpool.tile([128, 2, 4, cout], bf16, tag="m")
                    ab = 0
                else:
                    yst, m = pend
                    ab = 1
                ps = pspool.tile([128, 4, cout], f32, tag="ps")
                for t in range(4):
                    s, par = TSEL[t]
                    evw = egf[:, s, :].rearrange("p (x two) -> p x two", two=2)
                    for dh in range(3):
                        q0 = p0 + tiles_w * dh
                        nc.tensor.matmul(
                            ps[:, t, :],
                            evw[:, q0 : q0 + 128, par],
                            wt_t[:, dh, t, :],
                            start=(dh == 0),
                            stop=(dh == 2),
                        )
                nc.scalar.copy(m[:, ab, :, :], ps[:])
                if pend is None:
                    pend = (yst, m)
                else:
                    # inverse transform for both M-tiles of the pair (FD=512)
                    y0 = yst[:, :, 0, :]
                    y1 = yst[:, :, 1, :]
                    nc.vector.tensor_add(y0, m[:, :, 0, :], m[:, :, 1, :])
                    nc.vector.tensor_add(y0, y0, m[:, :, 2, :])
                    nc.vector.tensor_sub(y1, m[:, :, 1, :], m[:, :, 2, :])
                    nc.vector.tensor_sub(y1, y1, m[:, :, 3, :])
                    b0 = mt - 1  # pair covers global M-tiles mt-1, mt
                    dst = outv.rearrange("(b p) x -> b p x", p=128)[
                        b0 : b0 + 2
                    ].rearrange("b p x -> p b x")
                    nc.gpsimd.dma_start(
                        out=dst, in_=yst[:].rearrange("p b j c -> p b (j c)")
                    )
                    pend = None

    nc.compile()
    return nc


_cached_nc = None


def _get_program():
    global _cached_nc
    if _cached_nc is None:
        _cached_nc = _build_program(IMG_PER_CORE, H, W_DIM, CIN, COUT)
    return _cached_nc


def _prep_inputs(x, W):
    import ml_dtypes

    wq = np.sign(W.astype(np.float32))  # sign(0)=0 matches jnp.sign
    wt = np.empty((3, 4, CIN, COUT), np.float32)
    wt[:, 0] = wq[:, 0]
    wt[:, 1] = (wq[:, 0] + wq[:, 1] + wq[:, 2]) * 0.5
    wt[:, 2] = (wq[:, 0] - wq[:, 1] + wq[:, 2]) * 0.5
    wt[:, 3] = wq[:, 2]
    # [3,4,cin,cout] -> [cin, 3, 4, cout]; values exact in bf16
    wt = np.ascontiguousarray(wt.transpose(2, 0, 1, 3)).astype(
        ml_dtypes.bfloat16
    )
    xb = x.astype(ml_dtypes.bfloat16)  # round-to-nearest
    in_maps = []
    for c in range(N_CORES):
        xs = np.ascontiguousarray(xb[c * IMG_PER_CORE : (c + 1) * IMG_PER_CORE])
        in_maps.append({"x": xs, "wt": wt})
    return in_maps


def run(x, W, b, trace=False, tmpdir=None):
    from concourse import bass_utils

    if trace:
        # the agent image's antenv lacks axon_hooks; wire the NTFF profile
        # hook up manually so trace=True yields exec_time_ns + pftrace
        import sys, types

        if "antenv.axon_hooks" not in sys.modules:
            import antenv
            from trn_agent_boot.trn_boot import _ntff_profile_via_ctypes

            mod = types.ModuleType("antenv.axon_hooks")
            _hook = _ntff_profile_via_ctypes("/opt/axon/libaxon_pjrt.so")
            mod.get_axon_ntff_profile_hook = lambda: _hook
            sys.modules["antenv.axon_hooks"] = mod
            antenv.axon_hooks = mod

    nc = _get_program()
    in_maps = _prep_inputs(x, W)
    res = bass_utils.run_bass_kernel_spmd(
        nc, in_maps, list(range(N_CORES)), trace=trace, tmpdir=tmpdir
    )
    out = np.concatenate([res.results[i]["out"] for i in range(N_CORES)], axis=0)
    b = np.asarray(b, dtype=np.float32)
    if b.any():
        out = out + b  # exact; b == 0 in the reference setup
    return out, res


def kernel(x, W, b):
    out, _ = run(x, W, b, trace=False)
    return out
